# revision 32
# baseline (speedup 1.0000x reference)
"""Distributed Trainium2 Bass kernel for nn_Attention (GQA attention + LoRA + RoPE).

Sharding: tensor-parallel over heads across 8 NeuronCores.
  - core c owns Q heads 4c..4c+3 and KV head c (GQA group).
  - wq/wk/wv column-sharded; wo COLUMN-sharded (each core computes a
    512-column slice of the output over the full 4096 contraction, fed by an
    AllGather of all cores' per-head attention outputs).
  - LoRA is folded into wq/wv on the host (x@wq + (x@A)@B == x@(wq + A@B)).
  - 1/sqrt(HD) folded into wq.
  - RoPE pair permutation folded into wq/wk column order: within each head the
    even dims come first, odd dims second, so on-device RoPE is plain
    elementwise math on partition halves.

Everything the device computes is bf16-in/f32-accumulate.
"""

import sys
import types

import numpy as np
import ml_dtypes

import concourse.bass as bass
from concourse import bacc
import concourse.mybir as mybir
import concourse.tile as tile
from concourse.bass_utils import run_bass_kernel_spmd
from concourse.masks import make_identity


def _ensure_axon_hooks():
    """run_bass_kernel_spmd(trace=True) imports antenv.axon_hooks, which some
    images lack; install a no-op shim so a BASS_TRACE env var can't crash us."""
    try:
        import antenv
    except ImportError:
        return
    if "antenv.axon_hooks" in sys.modules:
        return
    try:
        from antenv import axon_hooks  # noqa: F401
        return
    except ImportError:
        pass
    mod = types.ModuleType("antenv.axon_hooks")
    mod._hook = None
    mod.set_axon_ntff_profile_hook = lambda h: setattr(mod, "_hook", h)
    mod.get_axon_ntff_profile_hook = lambda: mod._hook
    sys.modules["antenv.axon_hooks"] = mod
    antenv.axon_hooks = mod


_ensure_axon_hooks()

B, S, D = 2, 1024, 4096
H, KVH, HD = 32, 8, 128
NCORES = 8
HPC = H // NCORES            # 4 q heads per core
QCOLS = HPC * HD             # 512
T = B * S                    # 2048
P = 128
KT = D // P                  # 32 k tiles
NQ = 4                       # token quarters (512 tokens each)
QW = T // NQ                 # 512
SQC = 2                      # sq chunks per batch
STB = S // P                 # 8 st blocks per batch

FP32 = mybir.dt.float32
BF16 = mybir.dt.bfloat16
EXP = mybir.ActivationFunctionType.Exp

_COMPILED = {}
LAST_RESULTS = None


def _st_list(variant, sqc):
    """st blocks contributing to sq chunk sqc.
    Returns (st, flag): causal -> flag means diagonal-crossing (needs
    triangular zeroing of probs); general -> flag means mask preload."""
    out = []
    for st in range(STB):
        if variant == "causal":
            if st >= 4 * sqc + 4:
                continue  # fully masked
            flag = st >= 4 * sqc
        elif variant == "nomask":
            flag = False
        else:
            flag = True
        out.append((st, flag))
    return out


def _build(variant, debug=False):
    nc = bacc.Bacc(None)

    xt_e = nc.declare_dram_parameter("xt", [P, KT, T], BF16, isOutput=False)
    wq_e = nc.declare_dram_parameter("wq", [P, KT, QCOLS], BF16, isOutput=False)
    wk_e = nc.declare_dram_parameter("wk", [P, KT, HD], BF16, isOutput=False)
    wv_e = nc.declare_dram_parameter("wv", [P, KT, HD], BF16, isOutput=False)
    wo_e = nc.declare_dram_parameter("wo", [P, KT, QCOLS], BF16, isOutput=False)
    # cos: [c; c] duplicated halves.  sin: [s; -s] (negated bottom half).
    cos_e = nc.declare_dram_parameter("cos", [P, T], BF16, isOutput=False)
    sin_e = nc.declare_dram_parameter("sin", [P, T], BF16, isOutput=False)
    if variant == "general":
        mk_e = nc.declare_dram_parameter("mk", [P, STB, S], BF16, isOutput=False)
    out_e = nc.declare_dram_parameter("out", [QCOLS, T], FP32, isOutput=True)
    if debug:
        dq_e = nc.declare_dram_parameter("dq", [HPC * P, T], BF16, isOutput=True)
        dk_e = nc.declare_dram_parameter("dk", [P, T], BF16, isOutput=True)
        dv_e = nc.declare_dram_parameter("dv", [P, B * STB * P], BF16, isOutput=True)
        da_e = nc.declare_dram_parameter("da", [HPC * P, T], BF16, isOutput=True)
        dpr_e = nc.declare_dram_parameter("dpr", [4 * P, QW], BF16, isOutput=True)
        drb_e = nc.declare_dram_parameter("drb", [P, QW], FP32, isOutput=True)

    with tile.TileContext(nc) as tc:
        with (
            tc.tile_pool(name="wpool", bufs=1) as wpool,
            tc.tile_pool(name="cst", bufs=1) as cst,
            tc.tile_pool(name="persist", bufs=1) as persist,
            tc.tile_pool(name="xt", bufs=12) as xtp,
            tc.tile_pool(name="ev", bufs=5) as evp,
            tc.tile_pool(name="rt", bufs=4) as rtp,
            tc.tile_pool(name="probs", bufs=20) as prp,
            tc.tile_pool(name="misc", bufs=3) as mip,
            tc.tile_pool(name="ag", bufs=6) as agp,
            tc.tile_pool(name="ow", bufs=4) as owp,
            tc.tile_pool(name="ps", bufs=8, space="PSUM") as psp,
            tc.tile_pool(name="dram", bufs=1, space="DRAM") as dram,
        ):
            # ---- resident weights / constants ----
            wq_sb = wpool.tile([P, KT, QCOLS], BF16, name="wq_sb")
            wk_sb = wpool.tile([P, KT, HD], BF16, name="wk_sb")
            wv_sb = wpool.tile([P, KT, HD], BF16, name="wv_sb")
            wo_sb = wpool.tile([P, KT, QCOLS], BF16, name="wo_sb")
            cos_sb = wpool.tile([P, T], BF16, name="cos_sb")
            sin_sb = wpool.tile([P, T], BF16, name="sin_sb")
            if variant == "general":
                mk_sb = wpool.tile([P, STB, S], BF16, name="mk_sb")

            ident = cst.tile([P, P], BF16, name="ident")
            make_identity(nc, ident)
            ones_sq = cst.tile([P, P], BF16, name="ones_sq")
            nc.vector.memset(ones_sq[:], 1.0)

            # ---- persistent activations ----
            q_rot = [[persist.tile([P, S], BF16, name=f"q{h}_{b}")
                      for b in range(B)] for h in range(HPC)]
            k_rot = [persist.tile([P, S], BF16, name=f"k{b}") for b in range(B)]
            v_sb = [persist.tile([P, STB, P], BF16, name=f"v{b}") for b in range(B)]
            attn = [[persist.tile([P, S], BF16, name=f"attn{h}_{b}")
                     for b in range(B)] for h in range(HPC)]

            ag_in = [dram.tile([HPC * P, S], BF16, name=f"agin{b}") for b in range(B)]
            ag_out = [dram.tile([H * P, S], BF16, addr_space="Shared",
                                name=f"agout{b}") for b in range(B)]

            def rope(dst, dst_off, src_bf, qoff):
                """RoPE on split layout (a=0:64, b=64:128).
                p1 = [a*c; b*c];  p2 = [a*s; -b*s];  swap halves of p2;
                dst = p1 + p2sw = [a*c - b*s; a*s + b*c]."""
                c = cos_sb[:, qoff:qoff + QW]
                s = sin_sb[:, qoff:qoff + QW]
                p1 = rtp.tile([P, QW], BF16, name="p1")
                p2 = rtp.tile([P, QW], BF16, name="p2")
                p2sw = rtp.tile([P, QW], BF16, name="p2sw")
                nc.vector.tensor_mul(p1[:], src_bf[:], c)
                nc.vector.tensor_mul(p2[:], src_bf[:], s)
                nc.vector.tensor_copy(p2sw[0:64, :], p2[64:128, :])
                nc.vector.tensor_copy(p2sw[64:128, :], p2[0:64, :])
                nc.vector.tensor_add(dst[:, dst_off:dst_off + QW], p1[:], p2sw[:])

            def load_rope_tables():
                nc.gpsimd.dma_start(cos_sb[:], cos_e[:])
                nc.gpsimd.dma_start(sin_sb[:], sin_e[:])
                if variant == "general":
                    nc.gpsimd.dma_start(mk_sb[:], mk_e[:])

            def proj_quarter(qx):
                b, boff = qx // 2, (qx % 2) * QW
                toff = qx * QW
                # mb order: k, v, q0..q3 so k/v rope+transpose overlap q matmuls
                psums = [psp.tile([P, QW], FP32, name="ps", tag="ps")
                         for _ in range(6)]
                for k in range(KT):
                    if qx == 0:
                        nc.sync.dma_start(wk_sb[:, k, :], wk_e[:, k, :])
                        nc.sync.dma_start(wv_sb[:, k, :], wv_e[:, k, :])
                        nc.sync.dma_start(wq_sb[:, k, :], wq_e[:, k, :])
                    xt = xtp.tile([P, QW], BF16, name="xt")
                    nc.sync.dma_start(xt[:], xt_e[:, k, toff:toff + QW])
                    for mb in range(6):
                        if mb == 0:
                            w = wk_sb[:, k, :]
                        elif mb == 1:
                            w = wv_sb[:, k, :]
                        else:
                            w = wq_sb[:, k, (mb - 2) * P:(mb - 1) * P]
                        nc.tensor.matmul(psums[mb][:], w, xt[:],
                                         start=(k == 0), stop=(k == KT - 1))
                # evictions: k first, then v (transpose), then q heads
                ke = evp.tile([P, QW], BF16, name="ke", tag="qe")
                nc.scalar.copy(ke[:], psums[0][:])
                rope(k_rot[b], boff, ke, toff)
                ve = evp.tile([P, QW], BF16, name="ve", tag="qe")
                nc.scalar.copy(ve[:], psums[1][:])
                for i in range(QW // P):
                    st = (qx % 2) * 4 + i
                    tp = psp.tile([P, P], BF16, name="tp", tag="ps")
                    nc.tensor.transpose(tp[:], ve[:, i * P:(i + 1) * P], ident[:])
                    nc.scalar.copy(v_sb[b][:, st, :], tp[:])
                for h in range(HPC):
                    qe = evp.tile([P, QW], BF16, name="qe", tag="qe")
                    nc.scalar.copy(qe[:], psums[2 + h][:])
                    rope(q_rot[h][b], boff, qe, toff)

            def attention_batch(b, sqcs=(0, 1)):
                for sqc in sqcs:
                    sq0 = sqc * QW
                    stl = _st_list(variant, sqc)
                    for h in range(HPC):
                        # scores^T + exp
                        prtiles = []
                        for st, flag in stl:
                            pss = psp.tile([P, QW], FP32, name="pss", tag="ps")
                            preload = flag and variant == "general"
                            if preload:
                                nc.tensor.matmul(pss[:], ident[:],
                                                 mk_sb[:, st, sq0:sq0 + QW],
                                                 start=True, stop=False)
                            nc.tensor.matmul(
                                pss[:], k_rot[b][:, st * P:(st + 1) * P],
                                q_rot[h][b][:, sq0:sq0 + QW],
                                start=(not preload), stop=True)
                            pr = prp.tile([P, QW], BF16, name="pr", tag="pr")
                            nc.scalar.activation(pr[:], pss[:], EXP)
                            if flag and variant == "causal":
                                # zero probs where st*128+p > sq0+q (future keys)
                                nc.gpsimd.affine_select(
                                    out=pr[:], in_=pr[:],
                                    compare_op=mybir.AluOpType.is_ge,
                                    fill=0.0,
                                    base=sq0 - st * P,
                                    channel_multiplier=-1,
                                    pattern=[[1, QW]])
                            prtiles.append(pr)
                        # PV and denominator (denom broadcast to all partitions)
                        pso = psp.tile([P, QW], FP32, name="pso", tag="ps")
                        psdb = psp.tile([P, QW], FP32, name="psdb", tag="ps")
                        for i, (st, _) in enumerate(stl):
                            nc.tensor.matmul(pso[:], v_sb[b][:, st, :],
                                             prtiles[i][:],
                                             start=(i == 0), stop=(i == len(stl) - 1))
                            nc.tensor.matmul(psdb[:], ones_sq[:],
                                             prtiles[i][:],
                                             start=(i == 0), stop=(i == len(stl) - 1))
                        rb = mip.tile([P, QW], FP32, name="rb")
                        nc.vector.reciprocal_approx_fast(rb[:], psdb[:])
                        nc.vector.tensor_mul(attn[h][b][:, sq0:sq0 + QW],
                                             pso[:], rb[:])
                        if sqc == SQC - 1:
                            # ship this head to the gather bounce buffer ASAP
                            # (gpsimd queue: don't block the xt/weight stream)
                            nc.gpsimd.dma_start(
                                ag_in[b][h * P:(h + 1) * P, :], attn[h][b][:])
                        if debug and b == 0 and sqc == 0 and h == 0:
                            for i, (st, _) in enumerate(stl):
                                nc.sync.dma_start(
                                    dpr_e[st * P:(st + 1) * P, :], prtiles[i][:])
                            nc.sync.dma_start(drb_e[:], rb[:])

            def gather_batch(b):
                nc.gpsimd.collective_compute(
                    "AllGather", mybir.AluOpType.bypass,
                    ins=[ag_in[b][:].opt()],
                    outs=[ag_out[b][:].opt()],
                    replica_groups=[list(range(NCORES))],
                )

            def wo_load():
                for k in range(KT):
                    nc.gpsimd.dma_start(wo_sb[:, k, :], wo_e[:, k, :])

            def wo_batch(b):
                ag_r = ag_out[b].rearrange("(k p) t -> p k t", p=P)
                dma = nc.gpsimd.dma_start
                mb_groups = [(0, 1, 2, 3)] if b == 0 else [(0, 1), (2, 3)]
                for mbs in mb_groups:
                    psw = {mb: [psp.tile([P, QW], FP32, name="psw", tag="ps")
                                for _ in range(SQC)] for mb in mbs}
                    for k in range(KT):
                        agt = agp.tile([P, S], BF16, name="agt")
                        dma(agt[:], ag_r[:, k, :])
                        for mb in mbs:
                            w = wo_sb[:, k, mb * P:(mb + 1) * P]
                            for nch in range(SQC):
                                nc.tensor.matmul(psw[mb][nch][:], w,
                                                 agt[:, nch * QW:(nch + 1) * QW],
                                                 start=(k == 0), stop=(k == KT - 1))
                    for mb in mbs:
                        for nch in range(SQC):
                            ow = owp.tile([P, QW], FP32, name="ow")
                            nc.scalar.copy(ow[:], psw[mb][nch][:])
                            nc.scalar.dma_start(
                                out_e[mb * P:(mb + 1) * P,
                                      b * S + nch * QW:b * S + (nch + 1) * QW],
                                ow[:])

            # ---- timeline ----
            load_rope_tables()
            proj_quarter(0)
            proj_quarter(1)
            attention_batch(0)
            gather_batch(0)
            proj_quarter(2)
            wo_load()
            proj_quarter(3)
            attention_batch(1)
            wo_batch(0)
            gather_batch(1)
            wo_batch(1)

            if debug:
                for h in range(HPC):
                    for b in range(B):
                        nc.sync.dma_start(
                            dq_e[h * P:(h + 1) * P, b * S:(b + 1) * S],
                            q_rot[h][b][:])
                        nc.sync.dma_start(
                            da_e[h * P:(h + 1) * P, b * S:(b + 1) * S],
                            attn[h][b][:])
                for b in range(B):
                    nc.sync.dma_start(dk_e[:, b * S:(b + 1) * S], k_rot[b][:])
                    nc.sync.dma_start(
                        dv_e[:, b * STB * P:(b + 1) * STB * P],
                        v_sb[b][:].rearrange("p a b -> p (a b)"))

    nc.compile()
    return nc


def _get_compiled(variant):
    if variant not in _COMPILED:
        _COMPILED[variant] = _build(variant)
    return _COMPILED[variant]


def _detect_variant(mask2d):
    if not np.any(mask2d):
        return "nomask"
    tril = np.tril(mask2d)
    if not np.any(tril):
        iu = np.triu_indices(S, 1)
        if np.all(mask2d[iu] <= -1e8):
            return "causal"
    return "general"


def _pack_kt(w):
    """[R*128, N] -> [128, R, N] so that [:, k, :] is rows k*128..k*128+127."""
    return np.ascontiguousarray(w.reshape(w.shape[0] // P, P, -1).transpose(1, 0, 2))


def kernel(x, wq, wk, wv, wo, lora_q_a, lora_q_b, lora_v_a, lora_v_b,
           freqs_cos, freqs_sin, mask, start_pos=0, **_):
    global LAST_RESULTS
    bf = ml_dtypes.bfloat16
    x = np.asarray(x, np.float32)
    wq = np.asarray(wq, np.float32)
    wk = np.asarray(wk, np.float32)
    wv = np.asarray(wv, np.float32)
    wo = np.asarray(wo, np.float32)
    lora_q_a = np.asarray(lora_q_a, np.float32)
    lora_q_b = np.asarray(lora_q_b, np.float32)
    lora_v_a = np.asarray(lora_v_a, np.float32)
    lora_v_b = np.asarray(lora_v_b, np.float32)
    cos = np.asarray(freqs_cos, np.float32)
    sin = np.asarray(freqs_sin, np.float32)
    mask2d = np.asarray(mask, np.float32).reshape(S, S)

    variant = _detect_variant(mask2d)
    nc = _get_compiled(variant)

    # fold LoRA + scale; permute rope pairs (evens then odds within each head)
    wq_eff = (wq + lora_q_a @ lora_q_b) * np.float32(1.0 / np.sqrt(HD))
    wv_eff = wv + lora_v_a @ lora_v_b
    perm = np.concatenate([np.arange(0, HD, 2), np.arange(1, HD, 2)])
    qperm = (np.arange(H)[:, None] * HD + perm[None, :]).reshape(-1)
    kperm = (np.arange(KVH)[:, None] * HD + perm[None, :]).reshape(-1)
    wq_eff = wq_eff[:, qperm]
    wk_p = wk[:, kperm]

    xt = np.ascontiguousarray(x.reshape(T, D).T)        # [4096, 2048]
    xt_p = _pack_kt(xt).astype(bf)
    c64 = np.tile(cos.T, (1, B))                        # [64, 2048]
    s64 = np.tile(sin.T, (1, B))
    cosT = np.concatenate([c64, c64], axis=0).astype(bf)   # [c; c]
    sinT = np.concatenate([s64, -s64], axis=0).astype(bf)  # [s; -s]

    if variant == "general":
        maskT = np.ascontiguousarray(mask2d.T)          # [st, sq]
        mk = _pack_kt(maskT).astype(bf)                 # [128, 8, 1024]
    else:
        mk = None

    in_maps = []
    for c in range(NCORES):
        im = {
            "xt": xt_p,
            "wq": _pack_kt(wq_eff[:, c * QCOLS:(c + 1) * QCOLS]).astype(bf),
            "wk": _pack_kt(wk_p[:, c * HD:(c + 1) * HD]).astype(bf),
            "wv": _pack_kt(wv_eff[:, c * HD:(c + 1) * HD]).astype(bf),
            "wo": _pack_kt(wo[:, c * QCOLS:(c + 1) * QCOLS]).astype(bf),
            "cos": cosT,
            "sin": sinT,
        }
        if mk is not None:
            im["mk"] = mk
        in_maps.append(im)

    res = run_bass_kernel_spmd(nc, in_maps, core_ids=list(range(NCORES)))
    LAST_RESULTS = res
    outT = np.concatenate([res.results[c]["out"] for c in range(NCORES)], axis=0)
    return np.ascontiguousarray(outT.T).reshape(B, S, D).astype(np.float32)
